# revision 100
# baseline (speedup 1.0000x reference)
"""Trainium2 Bass kernel for nn_Net_60052232733176 (gnn_message_passing).

Strategy (graph-data parallel, 8 cores):
  - 50 graphs of 1000 nodes; core c handles graph slots [7c, 7c+7) (padded
    to 1024 nodes/graph).
  - Host side re-encodes each graph's edge list as a dense bf16 multiplicity
    matrix cnt[1024,1024] and ships x / x^T in bf16 (pure preprocessing).
  - GAT attention uses the exact identity
        exp(lrelu(s)) = exp(0.2*es_u) * exp(0.2*ed_v) * exp(0.8*relu(s)),
    so the dense per-tile work is one DVE relu-sum, one Act exp (with the
    0.2*es bias folded in), and one DVE multiply by cnt; the exp(0.2*ed_v)
    factor is applied after aggregation via the softmax denominator scale.
  - Softmax denominators via ones-row stationary PE matmuls accumulated
    into a [1,1024] PSUM row, transposed back with 8 tiny PE transposes.
  - TopK pooling via the gpsimd KthLargest ISA op on negated scores
    (k_adj = n_valid-k-1 <= 510), quantile chosen to lerp midway between
    the smallest-excluded and largest-kept value; keep = (u <= tau) * mask.
  - Readout max via tree reduce, mean via PE ones-matmuls; final MLP
    batched over graphs with log-softmax via Newton iterations for ln.
  - Scheduling: per-graph stages emitted as Python generators; a sliding
    window interleaves two graphs' stages so engine queues stay fed.

Self-contained: hardcodes all shapes; no file reads.
"""
import os
import numpy as np

import concourse.bass as bass
import concourse.bacc as bacc
import concourse.mybir as mybir
import concourse.tile as tile
from concourse.bass_utils import run_bass_kernel_spmd
from concourse.masks import make_identity
from concourse import bass_isa

F32 = mybir.dt.float32
BF16 = mybir.dt.bfloat16
AF = mybir.ActivationFunctionType
OP = mybir.AluOpType
AX = mybir.AxisListType

P = 128
B, NPG, D, C = 50, 1000, 128, 10
NP_ = 1024            # padded nodes per graph
NT = NP_ // P         # 8 node tiles
NCORES = 8
G = 7                 # graph slots per core
K1, K2, K3 = 800, 640, 512
NV1, NV2, NV3 = 1000, 800, 640   # live counts entering pool 1/2/3
BIGM = 100.0          # dead-node fold added to es before exp
BIGS = 1.0e30         # dead-node fold for topk scores / readout max

_cache = {}


# ----------------------------------------------------------------------------
# device program
# ----------------------------------------------------------------------------

def _build_program():
    KG = int(os.environ.get("K_GRAPHS", G))
    nc = bacc.Bacc(None, target_bir_lowering=False)

    # ---- DRAM tensors ----
    x_d = nc.dram_tensor("x_sh", [G, NP_, D], BF16, kind="ExternalInput")
    xT_d = nc.dram_tensor("xT_sh", [G, D, NP_], BF16, kind="ExternalInput")
    cnt_d = nc.dram_tensor("cnt_sh", [G, NP_, NP_], BF16, kind="ExternalInput")
    m0_d = nc.dram_tensor("m0", [P, NT], F32, kind="ExternalInput")

    def wparam(name, shape):
        return nc.dram_tensor(name, shape, F32, kind="ExternalInput")

    Wg_d = [wparam(f"W_g{l}", [D, D]) for l in (1, 2, 3)]
    asd_d = [wparam(f"asd_g{l}", [D, 2]) for l in (1, 2, 3)]
    bg_d = [wparam(f"b_g{l}", [D, 1]) for l in (1, 2, 3)]
    Wr_d = [wparam(f"Wr_c{l}", [D, D]) for l in (1, 2, 3)]
    br_d = [wparam(f"br_c{l}", [D, 1]) for l in (1, 2, 3)]
    Wo_d = [wparam(f"Wo_c{l}", [D, D]) for l in (1, 2, 3)]
    wp_d = {n: wparam(n, [D, 1]) for n in ("w_p20", "w_p30", "w_p11", "w_p21", "w_p31")}
    Wl1_d = wparam("W_l1", [2 * D, D])
    bl1_d = wparam("b_l1", [D, 1])
    Wl2_d = wparam("W_l2", [D, 64])
    bl2_d = wparam("b_l2", [64, 1])
    Wl3_d = wparam("W_l3", [64, C])
    bl3_d = wparam("b_l3", [C, 1])

    out_d = nc.dram_tensor("out", [G, C], F32, kind="ExternalOutput")
    KDBG = os.environ.get("K_DBG", "")
    dbg_d = nc.dram_tensor("dbg", [P, NP_], F32, kind="ExternalOutput") if KDBG else None

    with tile.TileContext(nc) as tc:
        import contextlib
        with contextlib.ExitStack() as ctx:
            cp = ctx.enter_context(tc.tile_pool(name="const", bufs=1))
            cbp = ctx.enter_context(tc.tile_pool(name="cntbf", bufs=3))
            Lp = ctx.enter_context(tc.tile_pool(name="Lp", bufs=6))
            stp = ctx.enter_context(tc.tile_pool(name="state", bufs=2))
            vp = ctx.enter_context(tc.tile_pool(name="vec", bufs=3))
            psA = ctx.enter_context(tc.tile_pool(name="psA", bufs=1, space="PSUM"))
            psD = ctx.enter_context(tc.tile_pool(name="psD", bufs=1, space="PSUM"))
            psT = ctx.enter_context(tc.tile_pool(name="psT", bufs=2, space="PSUM"))

            # ---- constants ----
            ident = cp.tile([P, P], F32, tag="ident")
            make_identity(nc, ident[:])
            ident_bf = cp.tile([P, P], BF16, tag="identbf")
            nc.vector.tensor_copy(out=ident_bf[:], in_=ident[:])
            ones_bf = cp.tile([P, 1], BF16, tag="onesbf")
            nc.vector.memset(ones_bf[:], 1.0)
            ones32 = cp.tile([P, 1], F32, tag="ones32")
            nc.vector.memset(ones32[:], 1.0)
            # PE warm: absorb gpsimd ident dep
            warm = psT.tile([P, P], F32, tag="pt")
            nc.tensor.transpose(out=warm[:], in_=ident[:], identity=ident[:])

            def load_w(dram, shape, tag, dtype=F32):
                t = cp.tile(shape, dtype, tag=tag)
                if dtype == F32:
                    nc.sync.dma_start(out=t[:], in_=dram[:])
                else:
                    tmp = cp.tile(shape, F32, tag=tag + "_st")
                    nc.sync.dma_start(out=tmp[:], in_=dram[:])
                    nc.vector.tensor_copy(out=t[:], in_=tmp[:])
                return t

            def col_to_rep(col_ap, out_tile_slice):
                """replicate a [128,1] column across partitions into [128,128]."""
                ps = psT.tile([P, P], F32, tag="pt")
                nc.tensor.transpose(out=ps[:], in_=col_ap.to_broadcast([P, P]),
                                    identity=ident[:])
                nc.vector.tensor_copy(out=out_tile_slice, in_=ps[:])

            # prefetch the first graphs' inputs BEFORE the ~30 weight DMAs
            # (SWDGE gen on SP is ~1us each; cnt g0 is needed ~8us in)
            m0 = load_w(m0_d, [P, NT], "m0")

            def issue_loads(g):
                xbf = stp.tile([P, NP_], BF16, tag="xbf")
                xbf3 = xbf[:].rearrange("p (c d) -> p c d", d=D)
                x_in3 = x_d[g].rearrange("(c p) d -> p c d", p=P)
                nc.gpsimd.dma_start(out=xbf3[:, :, :], in_=x_in3[:, :, :])
                xT = stp.tile([P, NP_], BF16, tag="xT")
                nc.gpsimd.dma_start(out=xT[:], in_=xT_d[g][:, :])
                cnt_t = cbp.tile([P, NT, NP_], BF16, tag="cnt")
                for q in range(4):
                    nc.sync.dma_start(
                        out=cnt_t[:, 2 * q:2 * q + 2, :],
                        in_=cnt_d[g].rearrange("(t p) v -> p t v", p=P)[
                            :, 2 * q:2 * q + 2, :])
                return dict(g=g, cnt=cnt_t[:].rearrange("p t v -> p (t v)"),
                            hT=xT, m_gat=m0, z_bf=xbf, zT=xT, m_gc=m0)

            sts = {}
            sts[0] = issue_loads(0)
            sts[1] = issue_loads(1)

            Wg = [load_w(Wg_d[i], [D, D], f"Wg{i}", BF16) for i in range(3)]
            asd = [load_w(asd_d[i], [D, 2], f"asd{i}", BF16) for i in range(3)]
            bg_col = [load_w(bg_d[i], [D, 1], f"bg{i}") for i in range(3)]
            Wr_bf = [load_w(Wr_d[i], [D, D], f"Wr{i}", BF16) for i in range(3)]
            br_col = [load_w(br_d[i], [D, 1], f"br{i}") for i in range(3)]
            Wo_bf = [load_w(Wo_d[i], [D, D], f"Wo{i}", BF16) for i in range(3)]
            wp = {n: load_w(d, [D, 1], n) for n, d in wp_d.items()}
            Wl1a = cp.tile([D, D], F32, tag="Wl1a")
            nc.sync.dma_start(out=Wl1a[:], in_=Wl1_d[0:D, :])
            Wl1b = cp.tile([D, D], F32, tag="Wl1b")
            nc.sync.dma_start(out=Wl1b[:], in_=Wl1_d[D:2 * D, :])
            bl1 = load_w(bl1_d, [D, 1], "bl1")
            Wl2 = load_w(Wl2_d, [D, 64], "Wl2")
            bl2 = load_w(bl2_d, [64, 1], "bl2")
            Wl3 = load_w(Wl3_d, [64, C], "Wl3")
            bl3 = load_w(bl3_d, [C, 1], "bl3")

            bg_rep = []
            for i in range(3):
                tf = cp.tile([P, P], F32, tag=f"bgrepf{i}")
                col_to_rep(bg_col[i][:, 0:1], tf[:])
                t = cp.tile([P, P], BF16, tag=f"bgrep{i}")
                nc.vector.tensor_copy(out=t[:], in_=tf[:])
                bg_rep.append(t)
            wrep = {}
            for n in wp:
                tf = cp.tile([P, P], F32, tag=f"repf_{n}")
                col_to_rep(wp[n][:, 0:1], tf[:])
                t = cp.tile([P, P], BF16, tag=f"rep_{n}")
                nc.vector.tensor_copy(out=t[:], in_=tf[:])
                wrep[n] = t

            # per-graph readout accumulators
            gacc0 = []
            gacc1 = []
            for g in range(G):
                ga = cp.tile([P, 1], F32, tag=f"gacc0_{g}")
                gb = cp.tile([P, 1], F32, tag=f"gacc1_{g}")
                gacc0.append(ga)
                gacc1.append(gb)
            for g in range(G):
                nc.vector.memset(gacc0[g][:], 0.0)
                nc.vector.memset(gacc1[g][:], 0.0)

            CH = [slice(c * P, (c + 1) * P) for c in range(NT)]
            # per-pool (k, n_valid): gat and gc branches share these
            pools_gat = [("w_p20", K1, NV1), ("w_p20", K2, NV2), ("w_p30", K3, NV3)]
            pools_gc = [("w_p11", K1, NV1), ("w_p21", K2, NV2), ("w_p31", K3, NV3)]

            def chunk_tree_reduce(src, col_out, op):
                """src [128, 1024] node-major; col_out [128,1] = reduce."""
                t1 = vp.tile([P, 512], BF16, tag="rt1")
                nc.vector.tensor_tensor(out=t1[:], in0=src[:, :512], in1=src[:, 512:],
                                        op=op)
                nc.vector.tensor_tensor(out=t1[:, :256], in0=t1[:, :256],
                                        in1=t1[:, 256:], op=op)
                nc.vector.tensor_tensor(out=t1[:, :128], in0=t1[:, :128],
                                        in1=t1[:, 128:256], op=op)
                ps = psT.tile([P, P], BF16, tag="pt")
                nc.tensor.matmul(ps[:], t1[:, :128], ident_bf[:], is_transpose=True)
                nc.vector.tensor_reduce(out=col_out, in_=ps[:], axis=AX.X, op=op)

            def gat_layer(st, li, sfx="a"):
                """hT: feat-major bf16 [128,1024]. Sets st["ha"] (node-major
                [128,1024] bf16, pre-pool). Generator: yields between op groups."""
                cnt_bf, hT, m_cur = st["cnt"], st["hT"], st["m_gat"]
                # hW node-major bf16 via two PSUM halves
                hW_bf = stp.tile([P, NP_], BF16, tag="hW" + sfx)
                for h in range(2):
                    pw = psT.tile([P, 512], F32, tag="pt")
                    for c in range(4):
                        nc.tensor.matmul(pw[:, c * P:(c + 1) * P],
                                         hT[:, CH[4 * h + c]], Wg[li][:],
                                         start=True, stop=True)
                    nc.scalar.activation(out=hW_bf[:, h * 512:(h + 1) * 512],
                                         in_=pw[:], func=AF.Copy)
                    yield
                # e vectors: [es | ed] per chunk into one PSUM tile, one evac
                pe = psT.tile([P, 2 * NT], F32, tag="pt")
                for c in range(NT):
                    nc.tensor.matmul(pe[:, 2 * c:2 * c + 2], hT[:, CH[c]], asd[li][:],
                                     start=True, stop=True)
                est = vp.tile([P, 2 * NT], F32, tag="est" + sfx)
                nc.vector.tensor_copy(out=est[:], in_=pe[:])
                yield
                est3 = est[:].rearrange("p (c two) -> p c two", two=2)
                est_e = est3[:, :, 0:1].rearrange("p c one -> p (c one)")
                est_o = est3[:, :, 1:2].rearrange("p c one -> p (c one)")
                # es2 = es + (m-1)*BIGM ; es2b = 0.2*es2 (exp bias)
                mf = vp.tile([P, NT], F32, tag="mf" + sfx)
                nc.vector.tensor_scalar(out=mf[:], in0=m_cur[:], scalar1=1.0,
                                        scalar2=BIGM, op0=OP.subtract, op1=OP.mult)
                es2 = vp.tile([P, NT], F32, tag="es2" + sfx)
                nc.vector.tensor_tensor(out=es2[:], in0=est_e, in1=mf[:], op=OP.add)
                es2b = vp.tile([P, NT], F32, tag="es2b" + sfx)
                nc.vector.tensor_scalar(out=es2b[:], in0=es2[:], scalar1=0.2,
                                        scalar2=None, op0=OP.mult)
                yield
                # ed_rep [128, 1024] bf16 via two PSUM halves
                ed_rep = stp.tile([P, NP_], BF16, tag="edr" + sfx)
                for h in range(2):
                    pw = psT.tile([P, 512], F32, tag="pt")
                    for c in range(4):
                        cc = 4 * h + c
                        nc.tensor.transpose(
                            out=pw[:, c * P:(c + 1) * P],
                            in_=est_o[:, cc:cc + 1].to_broadcast([P, P]),
                            identity=ident[:])
                    nc.scalar.activation(out=ed_rep[:, h * 512:(h + 1) * 512],
                                         in_=pw[:], func=AF.Copy)
                    yield
                # lselfF = lself / exp(0.2*ed) = exp(prelu(es2+ed) - 0.2*ed):
                # with this form rdF = 1/(den + lselfF) and csel = lselfF*rdF
                # directly — shortens the post-den critical tail by 2 ops and
                # removes the separate Fdn exp.
                eo2 = vp.tile([P, NT], F32, tag="eo2" + sfx)
                nc.vector.tensor_scalar(out=eo2[:], in0=est_o, scalar1=0.2,
                                        scalar2=None, op0=OP.mult)
                lselfF = vp.tile([P, NT], F32, tag="ls" + sfx)
                nc.vector.tensor_tensor(out=lselfF[:], in0=es2[:], in1=est_o,
                                        op=OP.add)
                nc.scalar.activation(out=lselfF[:], in_=lselfF[:], func=AF.Prelu,
                                     alpha=0.2)
                nc.vector.tensor_tensor(out=lselfF[:], in0=lselfF[:], in1=eo2[:],
                                        op=OP.subtract)
                nc.scalar.activation(out=lselfF[:], in_=lselfF[:], func=AF.Exp)
                yield
                # L tiles: relu-sum (DVE) -> exp(0.8*x + 0.2*es2) (Act)
                # -> * cnt (DVE); agg + den matmuls accumulate over t
                agg_ps = psA.tile([P, NP_], F32, tag="agA")
                den_ps = psD.tile([1, NP_], F32, tag="dnA")
                # software pipeline: relu-sum/exp run 2 tiles ahead of the
                # cnt-fold + matmuls so the DVE queue never head-of-line
                # blocks on Act's exp.
                Lts = []

                def emit_pre(t):
                    # q = max(0.8*(ed+es), 0); exp(q + 0.2*es)
                    Lt = Lp.tile([P, NP_], BF16, tag="L")
                    nc.vector.tensor_scalar(out=Lt[:], in0=ed_rep[:],
                                            scalar1=es2[:, t:t + 1], scalar2=0.0,
                                            op0=OP.add, op1=OP.max)
                    nc.scalar.activation(out=Lt[:], in_=Lt[:], func=AF.Exp,
                                         scale=0.8, bias=es2b[:, t:t + 1])
                    Lts.append(Lt)

                emit_pre(0)
                emit_pre(1)
                yield
                for t in range(NT):
                    if t + 2 < NT:
                        emit_pre(t + 2)
                    Lt = Lts[t]
                    nc.vector.tensor_tensor(out=Lt[:], in0=Lt[:],
                                            in1=cnt_bf[:, t * NP_:(t + 1) * NP_],
                                            op=OP.mult)
                    for h in range(2):
                        hs = slice(h * 512, (h + 1) * 512)
                        nc.tensor.matmul(agg_ps[:, hs], hW_bf[:, CH[t]], Lt[:, hs],
                                         start=(t == 0), stop=(t == NT - 1))
                        nc.tensor.matmul(den_ps[0:1, hs], ones_bf[:], Lt[:, hs],
                                         start=(t == 0), stop=(t == NT - 1))
                    yield
                # den row -> SBUF -> [128, NT] via tiny PE transposes
                den_row = vp.tile([1, NP_], F32, tag="dr" + sfx)
                pdc = psT.tile([P, NT], F32, tag="pt")
                nc.vector.tensor_copy(out=den_row[:, :512], in_=den_ps[0:1, :512])
                for c in range(4):
                    nc.tensor.transpose(out=pdc[:, c:c + 1],
                                        in_=den_row[0:1, CH[c]],
                                        identity=ident[0:1, 0:1])
                nc.vector.tensor_copy(out=den_row[:, 512:], in_=den_ps[0:1, 512:])
                for c in range(4, NT):
                    nc.tensor.transpose(out=pdc[:, c:c + 1],
                                        in_=den_row[0:1, CH[c]],
                                        identity=ident[0:1, 0:1])
                yield
                # rdF = 1/(den + lselfF); csel = lselfF * rdF  (see lselfF note)
                dtot = vp.tile([P, NT], F32, tag="dt" + sfx)
                nc.vector.tensor_tensor(out=dtot[:], in0=pdc[:], in1=lselfF[:],
                                        op=OP.add)
                rdF = vp.tile([P, NT], F32, tag="rdF" + sfx)
                nc.vector.reciprocal(out=rdF[:], in_=dtot[:])
                csel = vp.tile([P, NT], F32, tag="cs" + sfx)
                nc.vector.tensor_tensor(out=csel[:], in0=lselfF[:], in1=rdF[:],
                                        op=OP.mult)
                yield
                # evacuate agg (feat-major) to SBUF for transpose
                outT_sb = stp.tile([P, NP_], F32, tag="oT" + sfx)
                nc.scalar.activation(out=outT_sb[:], in_=agg_ps[:], func=AF.Copy)
                yield
                # finalize node-major: relu((aggT*rdF + hW*csel + b) * m)
                h_next = stp.tile([P, NP_], BF16, tag="hn" + sfx)
                for h in range(2):
                    pw = psT.tile([P, 512], F32, tag="pt")
                    for c in range(4):
                        nc.tensor.matmul(pw[:, c * P:(c + 1) * P],
                                         outT_sb[:, CH[4 * h + c]], ident[:],
                                         is_transpose=True)
                    for c in range(4):
                        cc = 4 * h + c
                        f1 = vp.tile([P, P], BF16, tag="f1" + sfx)
                        nc.vector.scalar_tensor_tensor(
                            out=f1[:], in0=hW_bf[:, CH[cc]],
                            scalar=csel[:, cc:cc + 1], in1=bg_rep[li][:],
                            op0=OP.mult, op1=OP.add)
                        f2 = vp.tile([P, P], BF16, tag="f2" + sfx)
                        nc.vector.scalar_tensor_tensor(
                            out=f2[:], in0=pw[:, c * P:(c + 1) * P],
                            scalar=rdF[:, cc:cc + 1], in1=f1[:],
                            op0=OP.mult, op1=OP.add)
                        nc.vector.tensor_scalar(
                            out=h_next[:, CH[cc]], in0=f2[:],
                            scalar1=m_cur[:, cc:cc + 1], scalar2=0.0,
                            op0=OP.mult, op1=OP.max)
                    yield
                if KDBG == f"ha{li}" and st["g"] == 0 and not st.get("_dbgd"):
                    st["_dbgd"] = True
                    dbf = vp.tile([P, NP_], F32, tag="dbf")
                    nc.vector.tensor_copy(out=dbf[:], in_=h_next[:])
                    nc.sync.dma_start(out=dbg_d[:], in_=dbf[:])
                st["ha"] = h_next

            def gc_layer(st, li, sfx="b"):
                """GraphConv: relu((lin_rel(sum_src z) + lin_root(z)) * m).
                Sets st["hb"]. Generator."""
                cnt_bf, z_bf, zT, m_cur = st["cnt"], st["z_bf"], st["zT"], st["m_gc"]
                agg_ps = psA.tile([P, NP_], F32, tag="agB")
                for t in range(NT):
                    for h in range(2):
                        nc.tensor.matmul(
                            agg_ps[:, h * 512:(h + 1) * 512],
                            z_bf[:, CH[t]],
                            cnt_bf[:, t * NP_ + h * 512: t * NP_ + (h + 1) * 512],
                            start=(t == 0), stop=(t == NT - 1))
                    if t % 2 == 1:
                        yield
                aggT_bf = stp.tile([P, NP_], BF16, tag="agb" + sfx)
                nc.scalar.activation(out=aggT_bf[:], in_=agg_ps[:], func=AF.Copy)
                yield
                outT_ps = agg_ps  # reuse the same PSUM tile (agg already evacuated)
                for h in range(2):
                    sl = slice(h * 512, (h + 1) * 512)
                    nc.tensor.matmul(outT_ps[:, sl], Wr_bf[li][:], aggT_bf[:, sl],
                                     start=True, stop=False)
                    nc.tensor.matmul(outT_ps[:, sl], Wo_bf[li][:], zT[:, sl],
                                     start=False, stop=True)
                    yield
                # + bias (per-feature = per-partition in feat-major); split
                # halves: each PSUM half stops separately, so the first
                # transpose batch can start before the second half evacuates
                outT_sb = stp.tile([P, NP_], F32, tag="oT" + sfx)
                nc.scalar.activation(out=outT_sb[:, :512], in_=outT_ps[:, :512],
                                     func=AF.Identity, bias=br_col[li][:, 0:1])
                nc.scalar.activation(out=outT_sb[:, 512:], in_=outT_ps[:, 512:],
                                     func=AF.Identity, bias=br_col[li][:, 0:1])
                yield
                h_next = stp.tile([P, NP_], BF16, tag="hn" + sfx)
                for h in range(2):
                    pw = psT.tile([P, 512], F32, tag="pt")
                    for c in range(4):
                        nc.tensor.matmul(pw[:, c * P:(c + 1) * P],
                                         outT_sb[:, CH[4 * h + c]], ident[:],
                                         is_transpose=True)
                    for c in range(4):
                        cc = 4 * h + c
                        if c % 2 == 0:
                            # mask-relu on Act: relu(pw * m) per-partition scale
                            nc.scalar.activation(
                                out=h_next[:, CH[cc]], in_=pw[:, c * P:(c + 1) * P],
                                func=AF.Relu, scale=m_cur[:, cc:cc + 1])
                        else:
                            nc.vector.tensor_scalar(
                                out=h_next[:, CH[cc]], in0=pw[:, c * P:(c + 1) * P],
                                scalar1=m_cur[:, cc:cc + 1], scalar2=0.0,
                                op0=OP.mult, op1=OP.max)
                    yield
                st["hb"] = h_next

            def topk_core(st, li, sfx):
                """scores -> kth-largest threshold -> keep -> pooled h (+hT).
                Gates the next layer stage; readout moved to topk_tail."""
                if sfx == "a":
                    h_next, m_cur = st["ha"], st["m_gat"]
                    wn, k, nv = pools_gat[li]
                else:
                    h_next, m_cur = st["hb"], st["m_gc"]
                    wn, k, nv = pools_gc[li]
                wrep_t = wrep[wn]
                need_hT = li < 2
                kadj = nv - k - 1
                quant = 1.0 - (kadj + 0.5) / (nv - 1)
                # mask fold (dead -> -BIGS after negation)
                mf = vp.tile([P, NT], F32, tag="mfs" + sfx)
                nc.vector.tensor_scalar(out=mf[:], in0=m_cur[:], scalar1=1.0,
                                        scalar2=BIGS, op0=OP.subtract, op1=OP.mult)
                # scores via fused mult + free-dim accumulate
                s = vp.tile([P, NT], F32, tag="s" + sfx)
                jnk = vp.tile([P, P], BF16, tag="jk" + sfx)
                for c in range(NT):
                    nc.vector.scalar_tensor_tensor(
                        out=jnk[:], in0=h_next[:, CH[c]], scalar=1.0,
                        in1=wrep_t[:], op0=OP.mult, op1=OP.mult,
                        accum_out=s[:, c:c + 1])
                    if c % 2 == 1:
                        yield
                # u = -s + mf  (dead -> -1e30); kth largest of u = boundary
                u = vp.tile([P, NT], F32, tag="u" + sfx)
                nc.vector.scalar_tensor_tensor(out=u[:], in0=s[:], scalar=-1.0,
                                               in1=mf[:], op0=OP.mult, op1=OP.add)
                tau2 = vp.tile([1, 2], F32, tag="tau" + sfx)
                nc.gpsimd.kth_largest(tau2[:], u[:], n_per_lane=NT, k=kadj,
                                      quantile=quant)
                thr = vp.tile([P, 1], F32, tag="thr" + sfx)
                nc.gpsimd.partition_broadcast(thr[:], tau2[0:1, 0:1])
                # tanh needs only s: runs during the Pool ISA ops
                th = vp.tile([P, NT], F32, tag="th" + sfx)
                nc.scalar.activation(out=th[:], in_=s[:], func=AF.Tanh)
                yield
                # keep = (u <= thr) * m
                keep = vp.tile([P, NT], F32, tag="kp" + sfx)
                nc.vector.scalar_tensor_tensor(out=keep[:], in0=u[:],
                                               scalar=thr[:, 0:1], in1=m_cur[:],
                                               op0=OP.is_le, op1=OP.mult)
                # pool scale = tanh(s) * keep
                pool = vp.tile([P, NT], F32, tag="pl" + sfx)
                nc.vector.tensor_tensor(out=pool[:], in0=th[:], in1=keep[:],
                                        op=OP.mult)
                yield
                h_pool = stp.tile([P, NP_], BF16, tag="hp" + sfx)
                for c in range(NT):
                    peng = nc.gpsimd if c % 2 == 1 else nc.vector
                    peng.tensor_scalar(out=h_pool[:, CH[c]],
                                       in0=h_next[:, CH[c]],
                                       scalar1=pool[:, c:c + 1], scalar2=None,
                                       op0=OP.mult)
                    if c % 2 == 1:
                        yield
                # hT for the next layer: it gates the next gat/gc stage
                hT_pool = None
                if need_hT:
                    hT_pool = stp.tile([P, NP_], BF16, tag="hT" + sfx)
                    for h in range(2):
                        pw = psT.tile([P, 512], BF16, tag="pt")
                        for c in range(4):
                            nc.tensor.matmul(pw[:, c * P:(c + 1) * P],
                                             h_pool[:, CH[4 * h + c]], ident_bf[:],
                                             is_transpose=True)
                        nc.scalar.activation(out=hT_pool[:, h * 512:(h + 1) * 512],
                                             in_=pw[:], func=AF.Copy)
                        yield
                st["ro_" + sfx] = (st["ha"] if sfx == "a" else st["hb"],
                                   h_pool, pool, keep, k, hT_pool)
                if sfx == "a":
                    st["m_gat"] = keep
                    st["hT"] = hT_pool
                else:
                    st["m_gc"] = keep
                    st["zT"] = hT_pool
                    st["z_bf"] = h_pool

            def topk_tail(st, li, sfx):
                """deferred readout: hm, masked max, mean; fills the next
                tick's layer stage with independent work."""
                g = st["g"]
                h_next, h_pool, pool, keep, k, hT_pool = st["ro_" + sfx]
                mx = vp.tile([P, 1], F32, tag="mx" + sfx)
                if hT_pool is not None:
                    # masked max from the feat-major transpose already built
                    # for the next layer: dead/pad columns are exactly 0 and
                    # the true per-feature max over kept nodes is >= 0 w.p. 1
                    # (any kept node with a relu-zeroed feature or positive
                    # tanh pins it), so no -inf fold is needed. bf16 TT tree
                    # (2x mode) + small TR beats one full-width TR (no modes).
                    tm = vp.tile([P, 512], BF16, tag="tm" + sfx)
                    nc.vector.tensor_tensor(out=tm[:], in0=hT_pool[:, :512],
                                            in1=hT_pool[:, 512:], op=OP.max)
                    nc.vector.tensor_tensor(out=tm[:, :256], in0=tm[:, :256],
                                            in1=tm[:, 256:], op=OP.max)
                    nc.vector.tensor_tensor(out=tm[:, :128], in0=tm[:, :128],
                                            in1=tm[:, 128:256], op=OP.max)
                    nc.vector.tensor_reduce(out=mx[:], in_=tm[:, :128],
                                            axis=AX.X, op=OP.max)
                else:
                    kf = vp.tile([P, NT], F32, tag="kf" + sfx)
                    nc.vector.tensor_scalar(out=kf[:], in0=keep[:], scalar1=1.0,
                                            scalar2=BIGS, op0=OP.subtract,
                                            op1=OP.mult)
                    hm = stp.tile([P, NP_], BF16, tag="hm" + sfx)
                    for c in range(NT):
                        nc.gpsimd.tensor_scalar(out=hm[:, CH[c]],
                                                in0=h_next[:, CH[c]],
                                                scalar1=pool[:, c:c + 1],
                                                scalar2=kf[:, c:c + 1],
                                                op0=OP.mult, op1=OP.add)
                        if c % 2 == 1:
                            yield
                    chunk_tree_reduce(hm[:], mx[:], OP.max)
                nc.vector.tensor_tensor(out=gacc0[g][:], in0=gacc0[g][:],
                                        in1=mx[:], op=OP.add)
                yield
                psm = psT.tile([P, 2], F32, tag="pt")
                for c in range(NT):
                    nc.tensor.matmul(psm[:, 0:1], h_pool[:, CH[c]], ones_bf[:],
                                     start=(c == 0), stop=(c == NT - 1))
                mn = vp.tile([P, 1], F32, tag="mn" + sfx)
                nc.vector.tensor_scalar(out=mn[:], in0=psm[:, 0:1], scalar1=1.0 / k,
                                        scalar2=None, op0=OP.mult)
                nc.vector.tensor_tensor(out=gacc1[g][:], in0=gacc1[g][:],
                                        in1=mn[:], op=OP.add)
                yield

            def drive(*streams):
                """round-robin the op streams until exhausted."""
                act = [iter(s) for s in streams if s is not None]
                while act:
                    for s in list(act):
                        try:
                            next(s)
                        except StopIteration:
                            act.remove(s)

            def chain(*gens):
                for gg in gens:
                    yield from gg

            SCHED = int(os.environ.get("K_SCHED", "0"))

            def stage_gen(st, idx):
                """graph tick idx: branch B trails branch A by one tick."""
                if idx == 0:
                    return [gat_layer(st, 0)]
                if idx == 6:
                    return [chain(topk_core(st, 2, "b"), topk_tail(st, 2, "b"))]
                li = (idx - 1) // 2
                if idx % 2 == 1:
                    return [chain(topk_core(st, li, "a"), topk_tail(st, li, "a")),
                            gc_layer(st, li)]
                return [gat_layer(st, li + 1),
                        chain(topk_core(st, li, "b"), topk_tail(st, li, "b"))]

            NTICK = 7
            OFF = int(os.environ.get("K_OFF", "3"))
            start = {g: OFF * g for g in range(KG)}
            nsteps = max(start[g] + NTICK for g in range(KG)) if KG else 0
            for s in range(nsteps):
                for g in range(KG):
                    if max(start[g] - 2, 0) == s and g not in sts:
                        sts[g] = issue_loads(g)
                streams = []
                for g in range(KG):
                    idx = s - start[g]
                    if 0 <= idx < NTICK:
                        streams.extend(stage_gen(sts[g], idx))
                drive(*streams)

            # ---- MLP over all graphs ----
            t1_ps = psT.tile([P, NT], F32, tag="pt")
            for g in range(G):
                nc.tensor.matmul(t1_ps[:, g:g + 1], Wl1a[:], gacc0[g][:],
                                 start=True, stop=False)
                nc.tensor.matmul(t1_ps[:, g:g + 1], Wl1b[:], gacc1[g][:],
                                 start=False, stop=True)
            t1 = vp.tile([P, G], F32, tag="t1")
            nc.vector.tensor_scalar(out=t1[:], in0=t1_ps[:, 0:G], scalar1=bl1[:, 0:1],
                                    scalar2=0.0, op0=OP.add, op1=OP.max)
            t2_ps = psT.tile([64, NT], F32, tag="pt")
            nc.tensor.matmul(t2_ps[:, 0:G], Wl2[:], t1[:], start=True, stop=True)
            t2p = vp.tile([64, G], F32, tag="t2p")
            nc.vector.tensor_scalar(out=t2p[:], in0=t2_ps[:, 0:G], scalar1=bl2[:, 0:1],
                                    scalar2=None, op0=OP.add)
            t2 = vp.tile([64, G], F32, tag="t2")
            nc.scalar.activation(out=t2[:], in_=t2p[:], func=AF.Prelu, alpha=0.01)
            t3_ps = psT.tile([C, 16], F32, tag="pt")
            nc.tensor.matmul(t3_ps[:, 0:G], Wl3[:], t2[:], start=True, stop=True)
            lg_cm = vp.tile([C, G], F32, tag="lgcm")
            nc.vector.tensor_scalar(out=lg_cm[:], in0=t3_ps[:, 0:G], scalar1=bl3[:, 0:1],
                                    scalar2=None, op0=OP.add)
            # transpose -> [G, C]
            lg_ps = psT.tile([G, 16], F32, tag="pt")
            nc.tensor.matmul(lg_ps[:, 0:C], lg_cm[:], ident[0:C, 0:C],
                             is_transpose=True)
            lg = vp.tile([G, C], F32, tag="lg")
            nc.vector.tensor_copy(out=lg[:], in_=lg_ps[:, 0:C])
            # log-sum-exp (logits are O(1))
            ex = vp.tile([G, C], F32, tag="ex")
            nc.scalar.activation(out=ex[:], in_=lg[:], func=AF.Exp)
            S = vp.tile([G, 1], F32, tag="S")
            nc.vector.tensor_reduce(out=S[:], in_=ex[:], axis=AX.X, op=OP.add)
            # ln(S) via Newton: y += S*exp(-y) - 1  (an Act Ln op would force
            # a 1283ns act-table-set switch — costlier than these small ops)
            y = vp.tile([G, 1], F32, tag="y")
            nc.vector.memset(y[:], 2.3)
            for _ in range(6):
                eny = vp.tile([G, 1], F32, tag="eny")
                nc.scalar.activation(out=eny[:], in_=y[:], func=AF.Exp, scale=-1.0)
                nc.vector.tensor_tensor(out=eny[:], in0=eny[:], in1=S[:], op=OP.mult)
                nc.vector.tensor_scalar(out=eny[:], in0=eny[:], scalar1=1.0,
                                        scalar2=None, op0=OP.subtract)
                nc.vector.tensor_tensor(out=y[:], in0=y[:], in1=eny[:], op=OP.add)
            outt = vp.tile([G, C], F32, tag="outt")
            nc.vector.tensor_scalar(out=outt[:], in0=lg[:], scalar1=y[:, 0:1],
                                    scalar2=None, op0=OP.subtract)
            nc.sync.dma_start(out=out_d[:], in_=outt[:])

    nc.compile()
    return nc


# ----------------------------------------------------------------------------
# host side
# ----------------------------------------------------------------------------

def _prep_in_maps(inputs):
    import ml_dtypes
    BF = ml_dtypes.bfloat16
    x = np.ascontiguousarray(np.asarray(inputs["x"], np.float32))
    ei = np.asarray(inputs["edge_index"]).astype(np.int64)
    src, dst = ei[0], ei[1]
    gid = src // NPG
    sl, dl = src % NPG, dst % NPG

    cnt = np.zeros((B, NP_, NP_), np.int8)
    np.add.at(cnt, (gid, sl, dl), 1)
    cnt = cnt.astype(BF)

    x_pad = np.zeros((B, NP_, D), np.float32)
    x_pad[:, :NPG] = x.reshape(B, NPG, D)
    x_pad = x_pad.astype(BF)

    m0 = np.zeros((NP_,), np.float32)
    m0[:NPG] = 1.0
    m0_packed = np.ascontiguousarray(m0.reshape(NT, P).T)  # [P, NT]

    def col(v):
        return np.ascontiguousarray(np.asarray(v, np.float32).reshape(-1, 1))

    weights = {}
    for l in (1, 2, 3):
        weights[f"W_g{l}"] = np.ascontiguousarray(np.asarray(inputs[f"W_g{l}"], np.float32))
        Wg = np.asarray(inputs[f"W_g{l}"], np.float32)
        weights[f"asd_g{l}"] = np.ascontiguousarray(
            Wg @ np.stack([np.asarray(inputs[f"as_g{l}"], np.float32),
                           np.asarray(inputs[f"ad_g{l}"], np.float32)], axis=1))
        weights[f"b_g{l}"] = col(inputs[f"b_g{l}"])
        weights[f"Wr_c{l}"] = np.ascontiguousarray(np.asarray(inputs[f"Wr_c{l}"], np.float32))
        weights[f"br_c{l}"] = col(inputs[f"br_c{l}"])
        weights[f"Wo_c{l}"] = np.ascontiguousarray(np.asarray(inputs[f"Wo_c{l}"], np.float32))
    for n in ("w_p20", "w_p30", "w_p11", "w_p21", "w_p31"):
        w = np.asarray(inputs[n], np.float32)
        weights[n] = col(w / np.linalg.norm(w))
    weights["W_l1"] = np.ascontiguousarray(np.asarray(inputs["W_l1"], np.float32))
    weights["b_l1"] = col(inputs["b_l1"])
    weights["W_l2"] = np.ascontiguousarray(np.asarray(inputs["W_l2"], np.float32))
    weights["b_l2"] = col(inputs["b_l2"])
    weights["W_l3"] = np.ascontiguousarray(np.asarray(inputs["W_l3"], np.float32))
    weights["b_l3"] = col(inputs["b_l3"])

    in_maps = []
    for c in range(NCORES):
        lo = c * G
        hi = min(lo + G, B)
        xs = np.zeros((G, NP_, D), BF)
        cs = np.zeros((G, NP_, NP_), BF)
        if hi > lo:
            xs[:hi - lo] = x_pad[lo:hi]
            cs[:hi - lo] = cnt[lo:hi]
        xTs = np.ascontiguousarray(xs.transpose(0, 2, 1))
        im = {"x_sh": xs, "xT_sh": xTs, "cnt_sh": cs, "m0": m0_packed}
        im.update(weights)
        in_maps.append(im)
    return in_maps


def kernel(**inputs) -> np.ndarray:
    if "nc" not in _cache:
        _cache["nc"] = _build_program()
    nc = _cache["nc"]
    in_maps = _prep_in_maps(inputs)
    res = run_bass_kernel_spmd(nc, in_maps, list(range(NCORES)))
    out = np.zeros((B, C), np.float32)
    for c in range(NCORES):
        lo = c * G
        hi = min(lo + G, B)
        if hi > lo:
            out[lo:hi] = np.asarray(res.results[c]["out"])[:hi - lo]
    return out


# revision 101
# speedup vs baseline: 1.0198x; 1.0198x over previous
"""Trainium2 Bass kernel for nn_Net_60052232733176 (gnn_message_passing).

Strategy (graph-data parallel, 8 cores):
  - 50 graphs of 1000 nodes; core c handles graph slots [7c, 7c+7) (padded
    to 1024 nodes/graph).
  - Host side re-encodes each graph's edge list as a dense bf16 multiplicity
    matrix cnt[1024,1024] and ships x / x^T in bf16 (pure preprocessing).
  - GAT attention uses the exact identity
        exp(lrelu(s)) = exp(0.2*es_u) * exp(0.2*ed_v) * exp(0.8*relu(s)),
    so the dense per-tile work is one DVE relu-sum, one Act exp (with the
    0.2*es bias folded in), and one DVE multiply by cnt; the exp(0.2*ed_v)
    factor is applied after aggregation via the softmax denominator scale.
  - Softmax denominators via ones-row stationary PE matmuls accumulated
    into a [1,1024] PSUM row, transposed back with 8 tiny PE transposes.
  - TopK pooling via the gpsimd KthLargest ISA op on negated scores
    (k_adj = n_valid-k-1 <= 510), quantile chosen to lerp midway between
    the smallest-excluded and largest-kept value; keep = (u <= tau) * mask.
  - Readout max via tree reduce, mean via PE ones-matmuls; final MLP
    batched over graphs with log-softmax via Newton iterations for ln.
  - Scheduling: per-graph stages emitted as Python generators; a sliding
    window interleaves two graphs' stages so engine queues stay fed.

Self-contained: hardcodes all shapes; no file reads.
"""
import os
import numpy as np

import concourse.bass as bass
import concourse.bacc as bacc
import concourse.mybir as mybir
import concourse.tile as tile
from concourse.bass_utils import run_bass_kernel_spmd
from concourse.masks import make_identity
from concourse import bass_isa

F32 = mybir.dt.float32
BF16 = mybir.dt.bfloat16
AF = mybir.ActivationFunctionType
OP = mybir.AluOpType
AX = mybir.AxisListType

P = 128
B, NPG, D, C = 50, 1000, 128, 10
NP_ = 1024            # padded nodes per graph
NT = NP_ // P         # 8 node tiles
NCORES = 8
G = 7                 # graph slots per core
K1, K2, K3 = 800, 640, 512
NV1, NV2, NV3 = 1000, 800, 640   # live counts entering pool 1/2/3
BIGM = 100.0          # dead-node fold added to es before exp
BIGS = 1.0e30         # dead-node fold for topk scores / readout max

_cache = {}


# ----------------------------------------------------------------------------
# device program
# ----------------------------------------------------------------------------

def _build_program():
    KG = int(os.environ.get("K_GRAPHS", G))
    nc = bacc.Bacc(None, target_bir_lowering=False)

    # ---- DRAM tensors ----
    x_d = nc.dram_tensor("x_sh", [G, NP_, D], BF16, kind="ExternalInput")
    xT_d = nc.dram_tensor("xT_sh", [G, D, NP_], BF16, kind="ExternalInput")
    cnt_d = nc.dram_tensor("cnt_sh", [G, NP_, NP_], BF16, kind="ExternalInput")
    m0_d = nc.dram_tensor("m0", [P, NT], F32, kind="ExternalInput")

    def wparam(name, shape):
        return nc.dram_tensor(name, shape, F32, kind="ExternalInput")

    Wg_d = [wparam(f"W_g{l}", [D, D]) for l in (1, 2, 3)]
    asd_d = [wparam(f"asd_g{l}", [D, 2]) for l in (1, 2, 3)]
    bg_d = [wparam(f"b_g{l}", [D, 1]) for l in (1, 2, 3)]
    Wr_d = [wparam(f"Wr_c{l}", [D, D]) for l in (1, 2, 3)]
    br_d = [wparam(f"br_c{l}", [D, 1]) for l in (1, 2, 3)]
    Wo_d = [wparam(f"Wo_c{l}", [D, D]) for l in (1, 2, 3)]
    wp_d = {n: wparam(n, [D, 1]) for n in ("w_p20", "w_p30", "w_p11", "w_p21", "w_p31")}
    Wl1_d = wparam("W_l1", [2 * D, D])
    bl1_d = wparam("b_l1", [D, 1])
    Wl2_d = wparam("W_l2", [D, 64])
    bl2_d = wparam("b_l2", [64, 1])
    Wl3_d = wparam("W_l3", [64, C])
    bl3_d = wparam("b_l3", [C, 1])

    out_d = nc.dram_tensor("out", [G, C], F32, kind="ExternalOutput")
    KDBG = os.environ.get("K_DBG", "")
    dbg_d = nc.dram_tensor("dbg", [P, NP_], F32, kind="ExternalOutput") if KDBG else None

    with tile.TileContext(nc) as tc:
        import contextlib
        with contextlib.ExitStack() as ctx:
            cp = ctx.enter_context(tc.tile_pool(name="const", bufs=1))
            cbp = ctx.enter_context(tc.tile_pool(name="cntbf", bufs=3))
            Lp = ctx.enter_context(tc.tile_pool(name="Lp", bufs=6))
            stp = ctx.enter_context(tc.tile_pool(name="state", bufs=2))
            vp = ctx.enter_context(tc.tile_pool(name="vec", bufs=3))
            psA = ctx.enter_context(tc.tile_pool(name="psA", bufs=1, space="PSUM"))
            psD = ctx.enter_context(tc.tile_pool(name="psD", bufs=1, space="PSUM"))
            psT = ctx.enter_context(tc.tile_pool(name="psT", bufs=2, space="PSUM"))

            # ---- constants ----
            ident = cp.tile([P, P], F32, tag="ident")
            make_identity(nc, ident[:])
            ident_bf = cp.tile([P, P], BF16, tag="identbf")
            nc.vector.tensor_copy(out=ident_bf[:], in_=ident[:])
            ones_bf = cp.tile([P, 1], BF16, tag="onesbf")
            nc.vector.memset(ones_bf[:], 1.0)
            ones32 = cp.tile([P, 1], F32, tag="ones32")
            nc.vector.memset(ones32[:], 1.0)
            # PE warm: absorb gpsimd ident dep
            warm = psT.tile([P, P], F32, tag="pt")
            nc.tensor.transpose(out=warm[:], in_=ident[:], identity=ident[:])

            def load_w(dram, shape, tag, dtype=F32):
                t = cp.tile(shape, dtype, tag=tag)
                if dtype == F32:
                    nc.sync.dma_start(out=t[:], in_=dram[:])
                else:
                    tmp = cp.tile(shape, F32, tag=tag + "_st")
                    nc.sync.dma_start(out=tmp[:], in_=dram[:])
                    nc.vector.tensor_copy(out=t[:], in_=tmp[:])
                return t

            def col_to_rep(col_ap, out_tile_slice):
                """replicate a [128,1] column across partitions into [128,128]."""
                ps = psT.tile([P, P], F32, tag="pt")
                nc.tensor.transpose(out=ps[:], in_=col_ap.to_broadcast([P, P]),
                                    identity=ident[:])
                nc.vector.tensor_copy(out=out_tile_slice, in_=ps[:])

            Wg = [load_w(Wg_d[i], [D, D], f"Wg{i}", BF16) for i in range(3)]
            asd = [load_w(asd_d[i], [D, 2], f"asd{i}", BF16) for i in range(3)]
            bg_col = [load_w(bg_d[i], [D, 1], f"bg{i}") for i in range(3)]
            Wr_bf = [load_w(Wr_d[i], [D, D], f"Wr{i}", BF16) for i in range(3)]
            br_col = [load_w(br_d[i], [D, 1], f"br{i}") for i in range(3)]
            Wo_bf = [load_w(Wo_d[i], [D, D], f"Wo{i}", BF16) for i in range(3)]
            wp = {n: load_w(d, [D, 1], n) for n, d in wp_d.items()}
            Wl1a = cp.tile([D, D], F32, tag="Wl1a")
            nc.sync.dma_start(out=Wl1a[:], in_=Wl1_d[0:D, :])
            Wl1b = cp.tile([D, D], F32, tag="Wl1b")
            nc.sync.dma_start(out=Wl1b[:], in_=Wl1_d[D:2 * D, :])
            bl1 = load_w(bl1_d, [D, 1], "bl1")
            Wl2 = load_w(Wl2_d, [D, 64], "Wl2")
            bl2 = load_w(bl2_d, [64, 1], "bl2")
            Wl3 = load_w(Wl3_d, [64, C], "Wl3")
            bl3 = load_w(bl3_d, [C, 1], "bl3")
            m0 = load_w(m0_d, [P, NT], "m0")

            bg_rep = []
            for i in range(3):
                tf = cp.tile([P, P], F32, tag=f"bgrepf{i}")
                col_to_rep(bg_col[i][:, 0:1], tf[:])
                t = cp.tile([P, P], BF16, tag=f"bgrep{i}")
                nc.vector.tensor_copy(out=t[:], in_=tf[:])
                bg_rep.append(t)
            wrep = {}
            for n in wp:
                tf = cp.tile([P, P], F32, tag=f"repf_{n}")
                col_to_rep(wp[n][:, 0:1], tf[:])
                t = cp.tile([P, P], BF16, tag=f"rep_{n}")
                nc.vector.tensor_copy(out=t[:], in_=tf[:])
                wrep[n] = t

            # per-graph readout accumulators
            gacc0 = []
            gacc1 = []
            for g in range(G):
                ga = cp.tile([P, 1], F32, tag=f"gacc0_{g}")
                gb = cp.tile([P, 1], F32, tag=f"gacc1_{g}")
                gacc0.append(ga)
                gacc1.append(gb)
            for g in range(G):
                nc.vector.memset(gacc0[g][:], 0.0)
                nc.vector.memset(gacc1[g][:], 0.0)

            CH = [slice(c * P, (c + 1) * P) for c in range(NT)]
            # per-pool (k, n_valid): gat and gc branches share these
            pools_gat = [("w_p20", K1, NV1), ("w_p20", K2, NV2), ("w_p30", K3, NV3)]
            pools_gc = [("w_p11", K1, NV1), ("w_p21", K2, NV2), ("w_p31", K3, NV3)]

            def chunk_tree_reduce(src, col_out, op):
                """src [128, 1024] node-major; col_out [128,1] = reduce."""
                t1 = vp.tile([P, 512], BF16, tag="rt1")
                nc.vector.tensor_tensor(out=t1[:], in0=src[:, :512], in1=src[:, 512:],
                                        op=op)
                nc.vector.tensor_tensor(out=t1[:, :256], in0=t1[:, :256],
                                        in1=t1[:, 256:], op=op)
                nc.vector.tensor_tensor(out=t1[:, :128], in0=t1[:, :128],
                                        in1=t1[:, 128:256], op=op)
                ps = psT.tile([P, P], BF16, tag="pt")
                nc.tensor.matmul(ps[:], t1[:, :128], ident_bf[:], is_transpose=True)
                nc.vector.tensor_reduce(out=col_out, in_=ps[:], axis=AX.X, op=op)

            def gat_layer(st, li, sfx="a"):
                """hT: feat-major bf16 [128,1024]. Sets st["ha"] (node-major
                [128,1024] bf16, pre-pool). Generator: yields between op groups."""
                cnt_bf, hT, m_cur = st["cnt"], st["hT"], st["m_gat"]
                # hW node-major bf16 via two PSUM halves
                hW_bf = stp.tile([P, NP_], BF16, tag="hW" + sfx)
                for h in range(2):
                    pw = psT.tile([P, 512], F32, tag="pt")
                    for c in range(4):
                        nc.tensor.matmul(pw[:, c * P:(c + 1) * P],
                                         hT[:, CH[4 * h + c]], Wg[li][:],
                                         start=True, stop=True)
                    nc.scalar.activation(out=hW_bf[:, h * 512:(h + 1) * 512],
                                         in_=pw[:], func=AF.Copy)
                    yield
                # e vectors: [es | ed] per chunk into one PSUM tile, one evac
                pe = psT.tile([P, 2 * NT], F32, tag="pt")
                for c in range(NT):
                    nc.tensor.matmul(pe[:, 2 * c:2 * c + 2], hT[:, CH[c]], asd[li][:],
                                     start=True, stop=True)
                est = vp.tile([P, 2 * NT], F32, tag="est" + sfx)
                nc.vector.tensor_copy(out=est[:], in_=pe[:])
                yield
                est3 = est[:].rearrange("p (c two) -> p c two", two=2)
                est_e = est3[:, :, 0:1].rearrange("p c one -> p (c one)")
                est_o = est3[:, :, 1:2].rearrange("p c one -> p (c one)")
                # es2 = es + (m-1)*BIGM ; es2b = 0.2*es2 (exp bias)
                mf = vp.tile([P, NT], F32, tag="mf" + sfx)
                nc.vector.tensor_scalar(out=mf[:], in0=m_cur[:], scalar1=1.0,
                                        scalar2=BIGM, op0=OP.subtract, op1=OP.mult)
                es2 = vp.tile([P, NT], F32, tag="es2" + sfx)
                nc.vector.tensor_tensor(out=es2[:], in0=est_e, in1=mf[:], op=OP.add)
                es2b = vp.tile([P, NT], F32, tag="es2b" + sfx)
                nc.vector.tensor_scalar(out=es2b[:], in0=es2[:], scalar1=0.2,
                                        scalar2=None, op0=OP.mult)
                yield
                # ed_rep [128, 1024] bf16 via two PSUM halves
                ed_rep = stp.tile([P, NP_], BF16, tag="edr" + sfx)
                for h in range(2):
                    pw = psT.tile([P, 512], F32, tag="pt")
                    for c in range(4):
                        cc = 4 * h + c
                        nc.tensor.transpose(
                            out=pw[:, c * P:(c + 1) * P],
                            in_=est_o[:, cc:cc + 1].to_broadcast([P, P]),
                            identity=ident[:])
                    nc.scalar.activation(out=ed_rep[:, h * 512:(h + 1) * 512],
                                         in_=pw[:], func=AF.Copy)
                    yield
                # lselfF = lself / exp(0.2*ed) = exp(prelu(es2+ed) - 0.2*ed):
                # with this form rdF = 1/(den + lselfF) and csel = lselfF*rdF
                # directly — shortens the post-den critical tail by 2 ops and
                # removes the separate Fdn exp.
                eo2 = vp.tile([P, NT], F32, tag="eo2" + sfx)
                nc.vector.tensor_scalar(out=eo2[:], in0=est_o, scalar1=0.2,
                                        scalar2=None, op0=OP.mult)
                lselfF = vp.tile([P, NT], F32, tag="ls" + sfx)
                nc.vector.tensor_tensor(out=lselfF[:], in0=es2[:], in1=est_o,
                                        op=OP.add)
                nc.scalar.activation(out=lselfF[:], in_=lselfF[:], func=AF.Prelu,
                                     alpha=0.2)
                nc.vector.tensor_tensor(out=lselfF[:], in0=lselfF[:], in1=eo2[:],
                                        op=OP.subtract)
                nc.scalar.activation(out=lselfF[:], in_=lselfF[:], func=AF.Exp)
                yield
                # L tiles: relu-sum (DVE) -> exp(0.8*x + 0.2*es2) (Act)
                # -> * cnt (DVE); agg + den matmuls accumulate over t
                agg_ps = psA.tile([P, NP_], F32, tag="agA")
                den_ps = psD.tile([1, NP_], F32, tag="dnA")
                # software pipeline: relu-sum/exp run 2 tiles ahead of the
                # cnt-fold + matmuls so the DVE queue never head-of-line
                # blocks on Act's exp.
                Lts = []

                def emit_pre(t):
                    # q = max(0.8*(ed+es), 0); exp(q + 0.2*es)
                    Lt = Lp.tile([P, NP_], BF16, tag="L")
                    nc.vector.tensor_scalar(out=Lt[:], in0=ed_rep[:],
                                            scalar1=es2[:, t:t + 1], scalar2=0.0,
                                            op0=OP.add, op1=OP.max)
                    nc.scalar.activation(out=Lt[:], in_=Lt[:], func=AF.Exp,
                                         scale=0.8, bias=es2b[:, t:t + 1])
                    Lts.append(Lt)

                emit_pre(0)
                emit_pre(1)
                yield
                for t in range(NT):
                    if t + 2 < NT:
                        emit_pre(t + 2)
                    Lt = Lts[t]
                    nc.vector.tensor_tensor(out=Lt[:], in0=Lt[:],
                                            in1=cnt_bf[:, t * NP_:(t + 1) * NP_],
                                            op=OP.mult)
                    for h in range(2):
                        hs = slice(h * 512, (h + 1) * 512)
                        nc.tensor.matmul(agg_ps[:, hs], hW_bf[:, CH[t]], Lt[:, hs],
                                         start=(t == 0), stop=(t == NT - 1))
                        nc.tensor.matmul(den_ps[0:1, hs], ones_bf[:], Lt[:, hs],
                                         start=(t == 0), stop=(t == NT - 1))
                    yield
                # den row -> SBUF -> [128, NT] via tiny PE transposes
                den_row = vp.tile([1, NP_], F32, tag="dr" + sfx)
                pdc = psT.tile([P, NT], F32, tag="pt")
                nc.vector.tensor_copy(out=den_row[:, :512], in_=den_ps[0:1, :512])
                for c in range(4):
                    nc.tensor.transpose(out=pdc[:, c:c + 1],
                                        in_=den_row[0:1, CH[c]],
                                        identity=ident[0:1, 0:1])
                nc.vector.tensor_copy(out=den_row[:, 512:], in_=den_ps[0:1, 512:])
                for c in range(4, NT):
                    nc.tensor.transpose(out=pdc[:, c:c + 1],
                                        in_=den_row[0:1, CH[c]],
                                        identity=ident[0:1, 0:1])
                yield
                # rdF = 1/(den + lselfF); csel = lselfF * rdF  (see lselfF note)
                dtot = vp.tile([P, NT], F32, tag="dt" + sfx)
                nc.vector.tensor_tensor(out=dtot[:], in0=pdc[:], in1=lselfF[:],
                                        op=OP.add)
                rdF = vp.tile([P, NT], F32, tag="rdF" + sfx)
                nc.vector.reciprocal(out=rdF[:], in_=dtot[:])
                csel = vp.tile([P, NT], F32, tag="cs" + sfx)
                nc.vector.tensor_tensor(out=csel[:], in0=lselfF[:], in1=rdF[:],
                                        op=OP.mult)
                yield
                # evacuate agg (feat-major) to SBUF for transpose
                outT_sb = stp.tile([P, NP_], F32, tag="oT" + sfx)
                nc.scalar.activation(out=outT_sb[:], in_=agg_ps[:], func=AF.Copy)
                yield
                # finalize node-major: relu((aggT*rdF + hW*csel + b) * m)
                h_next = stp.tile([P, NP_], BF16, tag="hn" + sfx)
                for h in range(2):
                    pw = psT.tile([P, 512], F32, tag="pt")
                    for c in range(4):
                        nc.tensor.matmul(pw[:, c * P:(c + 1) * P],
                                         outT_sb[:, CH[4 * h + c]], ident[:],
                                         is_transpose=True)
                    for c in range(4):
                        cc = 4 * h + c
                        f1 = vp.tile([P, P], BF16, tag="f1" + sfx)
                        nc.vector.scalar_tensor_tensor(
                            out=f1[:], in0=hW_bf[:, CH[cc]],
                            scalar=csel[:, cc:cc + 1], in1=bg_rep[li][:],
                            op0=OP.mult, op1=OP.add)
                        f2 = vp.tile([P, P], BF16, tag="f2" + sfx)
                        nc.vector.scalar_tensor_tensor(
                            out=f2[:], in0=pw[:, c * P:(c + 1) * P],
                            scalar=rdF[:, cc:cc + 1], in1=f1[:],
                            op0=OP.mult, op1=OP.add)
                        nc.vector.tensor_scalar(
                            out=h_next[:, CH[cc]], in0=f2[:],
                            scalar1=m_cur[:, cc:cc + 1], scalar2=0.0,
                            op0=OP.mult, op1=OP.max)
                    yield
                if KDBG == f"ha{li}" and st["g"] == 0 and not st.get("_dbgd"):
                    st["_dbgd"] = True
                    dbf = vp.tile([P, NP_], F32, tag="dbf")
                    nc.vector.tensor_copy(out=dbf[:], in_=h_next[:])
                    nc.sync.dma_start(out=dbg_d[:], in_=dbf[:])
                st["ha"] = h_next

            def gc_layer(st, li, sfx="b"):
                """GraphConv: relu((lin_rel(sum_src z) + lin_root(z)) * m).
                Sets st["hb"]. Generator."""
                cnt_bf, z_bf, zT, m_cur = st["cnt"], st["z_bf"], st["zT"], st["m_gc"]
                agg_ps = psA.tile([P, NP_], F32, tag="agB")
                for t in range(NT):
                    for h in range(2):
                        nc.tensor.matmul(
                            agg_ps[:, h * 512:(h + 1) * 512],
                            z_bf[:, CH[t]],
                            cnt_bf[:, t * NP_ + h * 512: t * NP_ + (h + 1) * 512],
                            start=(t == 0), stop=(t == NT - 1))
                    if t % 2 == 1:
                        yield
                aggT_bf = stp.tile([P, NP_], BF16, tag="agb" + sfx)
                nc.scalar.activation(out=aggT_bf[:], in_=agg_ps[:], func=AF.Copy)
                yield
                outT_ps = agg_ps  # reuse the same PSUM tile (agg already evacuated)
                for h in range(2):
                    sl = slice(h * 512, (h + 1) * 512)
                    nc.tensor.matmul(outT_ps[:, sl], Wr_bf[li][:], aggT_bf[:, sl],
                                     start=True, stop=False)
                    nc.tensor.matmul(outT_ps[:, sl], Wo_bf[li][:], zT[:, sl],
                                     start=False, stop=True)
                    yield
                # + bias (per-feature = per-partition in feat-major); split
                # halves: each PSUM half stops separately, so the first
                # transpose batch can start before the second half evacuates
                outT_sb = stp.tile([P, NP_], F32, tag="oT" + sfx)
                nc.scalar.activation(out=outT_sb[:, :512], in_=outT_ps[:, :512],
                                     func=AF.Identity, bias=br_col[li][:, 0:1])
                nc.scalar.activation(out=outT_sb[:, 512:], in_=outT_ps[:, 512:],
                                     func=AF.Identity, bias=br_col[li][:, 0:1])
                yield
                h_next = stp.tile([P, NP_], BF16, tag="hn" + sfx)
                for h in range(2):
                    pw = psT.tile([P, 512], F32, tag="pt")
                    for c in range(4):
                        nc.tensor.matmul(pw[:, c * P:(c + 1) * P],
                                         outT_sb[:, CH[4 * h + c]], ident[:],
                                         is_transpose=True)
                    for c in range(4):
                        cc = 4 * h + c
                        if c % 2 == 0:
                            # mask-relu on Act: relu(pw * m) per-partition scale
                            nc.scalar.activation(
                                out=h_next[:, CH[cc]], in_=pw[:, c * P:(c + 1) * P],
                                func=AF.Relu, scale=m_cur[:, cc:cc + 1])
                        else:
                            nc.vector.tensor_scalar(
                                out=h_next[:, CH[cc]], in0=pw[:, c * P:(c + 1) * P],
                                scalar1=m_cur[:, cc:cc + 1], scalar2=0.0,
                                op0=OP.mult, op1=OP.max)
                    yield
                st["hb"] = h_next

            def topk_core(st, li, sfx):
                """scores -> kth-largest threshold -> keep -> pooled h (+hT).
                Gates the next layer stage; readout moved to topk_tail."""
                if sfx == "a":
                    h_next, m_cur = st["ha"], st["m_gat"]
                    wn, k, nv = pools_gat[li]
                else:
                    h_next, m_cur = st["hb"], st["m_gc"]
                    wn, k, nv = pools_gc[li]
                wrep_t = wrep[wn]
                need_hT = li < 2
                kadj = nv - k - 1
                quant = 1.0 - (kadj + 0.5) / (nv - 1)
                # mask fold (dead -> -BIGS after negation)
                mf = vp.tile([P, NT], F32, tag="mfs" + sfx)
                nc.vector.tensor_scalar(out=mf[:], in0=m_cur[:], scalar1=1.0,
                                        scalar2=BIGS, op0=OP.subtract, op1=OP.mult)
                # scores via fused mult + free-dim accumulate
                s = vp.tile([P, NT], F32, tag="s" + sfx)
                jnk = vp.tile([P, P], BF16, tag="jk" + sfx)
                for c in range(NT):
                    nc.vector.scalar_tensor_tensor(
                        out=jnk[:], in0=h_next[:, CH[c]], scalar=1.0,
                        in1=wrep_t[:], op0=OP.mult, op1=OP.mult,
                        accum_out=s[:, c:c + 1])
                    if c % 2 == 1:
                        yield
                # u = -s + mf  (dead -> -1e30); kth largest of u = boundary
                u = vp.tile([P, NT], F32, tag="u" + sfx)
                nc.vector.scalar_tensor_tensor(out=u[:], in0=s[:], scalar=-1.0,
                                               in1=mf[:], op0=OP.mult, op1=OP.add)
                tau2 = vp.tile([1, 2], F32, tag="tau" + sfx)
                nc.gpsimd.kth_largest(tau2[:], u[:], n_per_lane=NT, k=kadj,
                                      quantile=quant)
                thr = vp.tile([P, 1], F32, tag="thr" + sfx)
                nc.gpsimd.partition_broadcast(thr[:], tau2[0:1, 0:1])
                # tanh needs only s: runs during the Pool ISA ops
                th = vp.tile([P, NT], F32, tag="th" + sfx)
                nc.scalar.activation(out=th[:], in_=s[:], func=AF.Tanh)
                yield
                # keep = (u <= thr) * m
                keep = vp.tile([P, NT], F32, tag="kp" + sfx)
                nc.vector.scalar_tensor_tensor(out=keep[:], in0=u[:],
                                               scalar=thr[:, 0:1], in1=m_cur[:],
                                               op0=OP.is_le, op1=OP.mult)
                # pool scale = tanh(s) * keep
                pool = vp.tile([P, NT], F32, tag="pl" + sfx)
                nc.vector.tensor_tensor(out=pool[:], in0=th[:], in1=keep[:],
                                        op=OP.mult)
                yield
                h_pool = stp.tile([P, NP_], BF16, tag="hp" + sfx)
                for c in range(NT):
                    peng = nc.gpsimd if c % 2 == 1 else nc.vector
                    peng.tensor_scalar(out=h_pool[:, CH[c]],
                                       in0=h_next[:, CH[c]],
                                       scalar1=pool[:, c:c + 1], scalar2=None,
                                       op0=OP.mult)
                    if c % 2 == 1:
                        yield
                # hT for the next layer: it gates the next gat/gc stage
                hT_pool = None
                if need_hT:
                    hT_pool = stp.tile([P, NP_], BF16, tag="hT" + sfx)
                    for h in range(2):
                        pw = psT.tile([P, 512], BF16, tag="pt")
                        for c in range(4):
                            nc.tensor.matmul(pw[:, c * P:(c + 1) * P],
                                             h_pool[:, CH[4 * h + c]], ident_bf[:],
                                             is_transpose=True)
                        nc.scalar.activation(out=hT_pool[:, h * 512:(h + 1) * 512],
                                             in_=pw[:], func=AF.Copy)
                        yield
                st["ro_" + sfx] = (st["ha"] if sfx == "a" else st["hb"],
                                   h_pool, pool, keep, k, hT_pool)
                if sfx == "a":
                    st["m_gat"] = keep
                    st["hT"] = hT_pool
                else:
                    st["m_gc"] = keep
                    st["zT"] = hT_pool
                    st["z_bf"] = h_pool

            def topk_tail(st, li, sfx):
                """deferred readout: hm, masked max, mean; fills the next
                tick's layer stage with independent work."""
                g = st["g"]
                h_next, h_pool, pool, keep, k, hT_pool = st["ro_" + sfx]
                mx = vp.tile([P, 1], F32, tag="mx" + sfx)
                if hT_pool is not None:
                    # masked max from the feat-major transpose already built
                    # for the next layer: dead/pad columns are exactly 0 and
                    # the true per-feature max over kept nodes is >= 0 w.p. 1
                    # (any kept node with a relu-zeroed feature or positive
                    # tanh pins it), so no -inf fold is needed. bf16 TT tree
                    # (2x mode) + small TR beats one full-width TR (no modes).
                    tm = vp.tile([P, 512], BF16, tag="tm" + sfx)
                    nc.vector.tensor_tensor(out=tm[:], in0=hT_pool[:, :512],
                                            in1=hT_pool[:, 512:], op=OP.max)
                    nc.vector.tensor_tensor(out=tm[:, :256], in0=tm[:, :256],
                                            in1=tm[:, 256:], op=OP.max)
                    nc.vector.tensor_tensor(out=tm[:, :128], in0=tm[:, :128],
                                            in1=tm[:, 128:256], op=OP.max)
                    nc.vector.tensor_reduce(out=mx[:], in_=tm[:, :128],
                                            axis=AX.X, op=OP.max)
                else:
                    kf = vp.tile([P, NT], F32, tag="kf" + sfx)
                    nc.vector.tensor_scalar(out=kf[:], in0=keep[:], scalar1=1.0,
                                            scalar2=BIGS, op0=OP.subtract,
                                            op1=OP.mult)
                    hm = stp.tile([P, NP_], BF16, tag="hm" + sfx)
                    for c in range(NT):
                        nc.gpsimd.tensor_scalar(out=hm[:, CH[c]],
                                                in0=h_next[:, CH[c]],
                                                scalar1=pool[:, c:c + 1],
                                                scalar2=kf[:, c:c + 1],
                                                op0=OP.mult, op1=OP.add)
                        if c % 2 == 1:
                            yield
                    chunk_tree_reduce(hm[:], mx[:], OP.max)
                nc.vector.tensor_tensor(out=gacc0[g][:], in0=gacc0[g][:],
                                        in1=mx[:], op=OP.add)
                yield
                psm = psT.tile([P, 2], F32, tag="pt")
                for c in range(NT):
                    nc.tensor.matmul(psm[:, 0:1], h_pool[:, CH[c]], ones_bf[:],
                                     start=(c == 0), stop=(c == NT - 1))
                mn = vp.tile([P, 1], F32, tag="mn" + sfx)
                nc.vector.tensor_scalar(out=mn[:], in0=psm[:, 0:1], scalar1=1.0 / k,
                                        scalar2=None, op0=OP.mult)
                nc.vector.tensor_tensor(out=gacc1[g][:], in0=gacc1[g][:],
                                        in1=mn[:], op=OP.add)
                yield

            def drive(*streams):
                """round-robin the op streams until exhausted."""
                act = [iter(s) for s in streams if s is not None]
                while act:
                    for s in list(act):
                        try:
                            next(s)
                        except StopIteration:
                            act.remove(s)

            def issue_loads(g):
                xbf = stp.tile([P, NP_], BF16, tag="xbf")
                xbf3 = xbf[:].rearrange("p (c d) -> p c d", d=D)
                x_in3 = x_d[g].rearrange("(c p) d -> p c d", p=P)
                nc.gpsimd.dma_start(out=xbf3[:, :, :], in_=x_in3[:, :, :])
                xT = stp.tile([P, NP_], BF16, tag="xT")
                nc.gpsimd.dma_start(out=xT[:], in_=xT_d[g][:, :])
                cnt_t = cbp.tile([P, NT, NP_], BF16, tag="cnt")
                for q in range(4):
                    nc.sync.dma_start(
                        out=cnt_t[:, 2 * q:2 * q + 2, :],
                        in_=cnt_d[g].rearrange("(t p) v -> p t v", p=P)[
                            :, 2 * q:2 * q + 2, :])
                return dict(g=g, cnt=cnt_t[:].rearrange("p t v -> p (t v)"),
                            hT=xT, m_gat=m0, z_bf=xbf, zT=xT, m_gc=m0)

            def chain(*gens):
                for gg in gens:
                    yield from gg

            SCHED = int(os.environ.get("K_SCHED", "0"))

            def stage_gen(st, idx):
                """graph tick idx: branch B trails branch A by one tick."""
                if idx == 0:
                    return [gat_layer(st, 0)]
                if idx == 6:
                    return [chain(topk_core(st, 2, "b"), topk_tail(st, 2, "b"))]
                li = (idx - 1) // 2
                if idx % 2 == 1:
                    return [chain(topk_core(st, li, "a"), topk_tail(st, li, "a")),
                            gc_layer(st, li)]
                return [gat_layer(st, li + 1),
                        chain(topk_core(st, li, "b"), topk_tail(st, li, "b"))]

            NTICK = 7
            OFF = int(os.environ.get("K_OFF", "3"))
            start = {g: OFF * g for g in range(KG)}
            nsteps = max(start[g] + NTICK for g in range(KG)) if KG else 0
            sts = {}
            for s in range(nsteps):
                for g in range(KG):
                    if max(start[g] - 2, 0) == s and g not in sts:
                        sts[g] = issue_loads(g)
                streams = []
                for g in range(KG):
                    idx = s - start[g]
                    if 0 <= idx < NTICK:
                        streams.extend(stage_gen(sts[g], idx))
                drive(*streams)

            # ---- MLP over all graphs ----
            t1_ps = psT.tile([P, NT], F32, tag="pt")
            for g in range(G):
                nc.tensor.matmul(t1_ps[:, g:g + 1], Wl1a[:], gacc0[g][:],
                                 start=True, stop=False)
                nc.tensor.matmul(t1_ps[:, g:g + 1], Wl1b[:], gacc1[g][:],
                                 start=False, stop=True)
            t1 = vp.tile([P, G], F32, tag="t1")
            nc.vector.tensor_scalar(out=t1[:], in0=t1_ps[:, 0:G], scalar1=bl1[:, 0:1],
                                    scalar2=0.0, op0=OP.add, op1=OP.max)
            t2_ps = psT.tile([64, NT], F32, tag="pt")
            nc.tensor.matmul(t2_ps[:, 0:G], Wl2[:], t1[:], start=True, stop=True)
            t2p = vp.tile([64, G], F32, tag="t2p")
            nc.vector.tensor_scalar(out=t2p[:], in0=t2_ps[:, 0:G], scalar1=bl2[:, 0:1],
                                    scalar2=None, op0=OP.add)
            t2 = vp.tile([64, G], F32, tag="t2")
            nc.scalar.activation(out=t2[:], in_=t2p[:], func=AF.Prelu, alpha=0.01)
            t3_ps = psT.tile([C, 16], F32, tag="pt")
            nc.tensor.matmul(t3_ps[:, 0:G], Wl3[:], t2[:], start=True, stop=True)
            lg_cm = vp.tile([C, G], F32, tag="lgcm")
            nc.vector.tensor_scalar(out=lg_cm[:], in0=t3_ps[:, 0:G], scalar1=bl3[:, 0:1],
                                    scalar2=None, op0=OP.add)
            # transpose -> [G, C]
            lg_ps = psT.tile([G, 16], F32, tag="pt")
            nc.tensor.matmul(lg_ps[:, 0:C], lg_cm[:], ident[0:C, 0:C],
                             is_transpose=True)
            lg = vp.tile([G, C], F32, tag="lg")
            nc.vector.tensor_copy(out=lg[:], in_=lg_ps[:, 0:C])
            # log-sum-exp (logits are O(1))
            ex = vp.tile([G, C], F32, tag="ex")
            nc.scalar.activation(out=ex[:], in_=lg[:], func=AF.Exp)
            S = vp.tile([G, 1], F32, tag="S")
            nc.vector.tensor_reduce(out=S[:], in_=ex[:], axis=AX.X, op=OP.add)
            # ln(S) via Newton: y += S*exp(-y) - 1  (an Act Ln op would force
            # a 1283ns act-table-set switch — costlier than these small ops)
            y = vp.tile([G, 1], F32, tag="y")
            nc.vector.memset(y[:], 2.3)
            for _ in range(6):
                eny = vp.tile([G, 1], F32, tag="eny")
                nc.scalar.activation(out=eny[:], in_=y[:], func=AF.Exp, scale=-1.0)
                nc.vector.tensor_tensor(out=eny[:], in0=eny[:], in1=S[:], op=OP.mult)
                nc.vector.tensor_scalar(out=eny[:], in0=eny[:], scalar1=1.0,
                                        scalar2=None, op0=OP.subtract)
                nc.vector.tensor_tensor(out=y[:], in0=y[:], in1=eny[:], op=OP.add)
            outt = vp.tile([G, C], F32, tag="outt")
            nc.vector.tensor_scalar(out=outt[:], in0=lg[:], scalar1=y[:, 0:1],
                                    scalar2=None, op0=OP.subtract)
            nc.sync.dma_start(out=out_d[:], in_=outt[:])

    nc.compile()
    return nc


# ----------------------------------------------------------------------------
# host side
# ----------------------------------------------------------------------------

def _prep_in_maps(inputs):
    import ml_dtypes
    BF = ml_dtypes.bfloat16
    x = np.ascontiguousarray(np.asarray(inputs["x"], np.float32))
    ei = np.asarray(inputs["edge_index"]).astype(np.int64)
    src, dst = ei[0], ei[1]
    gid = src // NPG
    sl, dl = src % NPG, dst % NPG

    cnt = np.zeros((B, NP_, NP_), np.int8)
    np.add.at(cnt, (gid, sl, dl), 1)
    cnt = cnt.astype(BF)

    x_pad = np.zeros((B, NP_, D), np.float32)
    x_pad[:, :NPG] = x.reshape(B, NPG, D)
    x_pad = x_pad.astype(BF)

    m0 = np.zeros((NP_,), np.float32)
    m0[:NPG] = 1.0
    m0_packed = np.ascontiguousarray(m0.reshape(NT, P).T)  # [P, NT]

    def col(v):
        return np.ascontiguousarray(np.asarray(v, np.float32).reshape(-1, 1))

    weights = {}
    for l in (1, 2, 3):
        weights[f"W_g{l}"] = np.ascontiguousarray(np.asarray(inputs[f"W_g{l}"], np.float32))
        Wg = np.asarray(inputs[f"W_g{l}"], np.float32)
        weights[f"asd_g{l}"] = np.ascontiguousarray(
            Wg @ np.stack([np.asarray(inputs[f"as_g{l}"], np.float32),
                           np.asarray(inputs[f"ad_g{l}"], np.float32)], axis=1))
        weights[f"b_g{l}"] = col(inputs[f"b_g{l}"])
        weights[f"Wr_c{l}"] = np.ascontiguousarray(np.asarray(inputs[f"Wr_c{l}"], np.float32))
        weights[f"br_c{l}"] = col(inputs[f"br_c{l}"])
        weights[f"Wo_c{l}"] = np.ascontiguousarray(np.asarray(inputs[f"Wo_c{l}"], np.float32))
    for n in ("w_p20", "w_p30", "w_p11", "w_p21", "w_p31"):
        w = np.asarray(inputs[n], np.float32)
        weights[n] = col(w / np.linalg.norm(w))
    weights["W_l1"] = np.ascontiguousarray(np.asarray(inputs["W_l1"], np.float32))
    weights["b_l1"] = col(inputs["b_l1"])
    weights["W_l2"] = np.ascontiguousarray(np.asarray(inputs["W_l2"], np.float32))
    weights["b_l2"] = col(inputs["b_l2"])
    weights["W_l3"] = np.ascontiguousarray(np.asarray(inputs["W_l3"], np.float32))
    weights["b_l3"] = col(inputs["b_l3"])

    in_maps = []
    for c in range(NCORES):
        lo = c * G
        hi = min(lo + G, B)
        xs = np.zeros((G, NP_, D), BF)
        cs = np.zeros((G, NP_, NP_), BF)
        if hi > lo:
            xs[:hi - lo] = x_pad[lo:hi]
            cs[:hi - lo] = cnt[lo:hi]
        xTs = np.ascontiguousarray(xs.transpose(0, 2, 1))
        im = {"x_sh": xs, "xT_sh": xTs, "cnt_sh": cs, "m0": m0_packed}
        im.update(weights)
        in_maps.append(im)
    return in_maps


def kernel(**inputs) -> np.ndarray:
    if "nc" not in _cache:
        _cache["nc"] = _build_program()
    nc = _cache["nc"]
    in_maps = _prep_in_maps(inputs)
    res = run_bass_kernel_spmd(nc, in_maps, list(range(NCORES)))
    out = np.zeros((B, C), np.float32)
    for c in range(NCORES):
        lo = c * G
        hi = min(lo + G, B)
        if hi > lo:
            out[lo:hi] = np.asarray(res.results[c]["out"])[:hi - lo]
    return out
